# revision 13
# baseline (speedup 1.0000x reference)
"""Bass/Tile kernel for the two-stage attention block (v4).

Layout (from v3): everything on-chip is transposed ([feature, token],
feature on partitions) so both attention stages chain with zero on-chip
transposes:

  QT/KT  = W.T @ x.T       : matmul(lhsT=W_chunk, rhs=xT_chunk)   -> [c, i]
  V      = x @ W           : matmul(lhsT=xT_chunk, rhs=Wv_chunk)  -> [j, c]
  S^T    = (q@k.T).T       : matmul(lhsT=KT_h, rhs=QT_h)          -> [j, i]
  P^T    = act(S^T * m^T)  : elementwise
  O^T    = (P@v).T         : matmul(lhsT=V_h, rhs=P^T_h)          -> [d, i]

v4 changes (driven by NTFF profiling: HAM clock-gate thrash + DVE
reciprocal chains + no PE-tile concurrency):
- Score matmuls for the two heads of a pair write one combined PSUM tile
  ([h_even 512 | h_odd 512]) and are emitted adjacently with alternating
  row groups (lhsT base partition 0/64) -> the PE array runs them
  concurrently (row tiling, ~2x measured).
- Stage-1 apply matmuls are col-tiled: h_even -> PSUM rows 0:64,
  h_odd -> rows 64:128 of the same tile (~2x), which also makes the
  eviction a single [128,512] copy and double-buffers apply PSUM across
  head pairs (no pair-boundary stall).
- Softmax denominators: reciprocal_approx_fast (~5x faster than
  reciprocal; 18 bits, plenty for bf16 data) and evictions emitted
  ic0-first so phase 5 / next pair unblock early.  This removes the
  5-12us PE-idle windows at pair boundaries that re-engaged the HAM
  clock gate (PE at 1.2GHz instead of 2.4GHz for ~40% of the kernel).
- Mask is pre-duplicated host-side ([j, ic0|ic0|ic1|ic1] layout) so the
  stage-1 mask multiply stays one [128,1024] DVE op per (j, ic).
"""

from contextlib import ExitStack

import concourse.bacc as bacc
import concourse.bass as bass
import concourse.tile as tile
from concourse import mybir
from concourse.vector_clock import ScopedClock

F32 = mybir.dt.float32
BF16 = mybir.dt.bfloat16
AF = mybir.ActivationFunctionType
ALU = mybir.AluOpType

N, DIM, H, D = 1024, 512, 8, 64
SCALE = DIM**-0.5
KC = DIM // 128  # contraction chunks for projections
JC = N // 128  # key-side chunks (128 wide)
IC = N // 512  # query-side chunks for 512-wide matmul outputs
EXP_BIAS = -15.0
VP = 2 * D  # per-head width in padded V2: 64 data cols + 64 ones cols
_STOP_PHASE = 99


# ---------------------------------------------------------------------------
# Walrus in this container rejects instructions with >1 sync-wait.
# Split: hoist extra waits onto single-wait NoOps inserted just before.
def legalize_single_wait(nc):
    n_split = 0
    for fn in nc.m.functions:
        for blk in fn.blocks:
            insts = list(blk.instructions)
            out = []
            changed = False
            for inst in insts:
                si = inst.sync_info
                waits = list(si.on_wait) if (si is not None and si.on_wait) else []
                if len(waits) > 1:
                    changed = True
                    n_split += len(waits) - 1
                    for w in waits[:-1]:
                        nop = mybir.InstNoOp(
                            name=nc.get_next_instruction_name(),
                            sync_info=mybir.SyncInfo(on_wait=[w], on_update=[]),
                            bass_nofuse=True,
                            engine=inst.engine,
                        )
                        nc.register_instruction(nop)
                        out.append(nop)
                    si.on_wait = [waits[-1]]
                out.append(inst)
            if changed:
                blk.instructions = out
    return n_split


def _patched_drain_and_barrier(self, tick_clock, wait_clock):
    drain_inst = self.nc.sync.drain()
    wait_clock.add_sem_waits(
        drain_inst.ins, ScopedClock({None: tick_clock.global_clock})
    )
    si = drain_inst.ins.sync_info
    waits = list(si.on_wait or []) if si is not None else []
    if len(waits) > 1:
        si.on_wait = [waits[0]]
        for w in waits[1:]:
            extra = self.nc.sync.drain()
            esi = extra.ins.sync_info
            if esi is None:
                extra.ins.sync_info = mybir.SyncInfo(on_wait=[w], on_update=[])
            else:
                esi.on_wait = [w]

    self.nc.all_engine_barrier()
    assert self.sems is not None
    popped = self.nc._tile_sem_poison_stack.pop()
    assert popped is self._sem_poison
    self.nc.clear_and_free_semaphores(list(self.sems.allocated().values()))
    self.nc.all_engine_barrier()


def install_patches():
    tile.TileContext._drain_and_barrier = _patched_drain_and_barrier


# ---------------------------------------------------------------------------


def build_prologue(ctx: ExitStack, tc: tile.TileContext, d, zb=False):
    """Load loop-invariant tensors (inputs, weights, mask, consts) once."""
    nc = tc.nc
    pool = ctx.enter_context(tc.tile_pool(name="persist", bufs=1))
    pre = {}

    pre["xT"] = [pool.tile([128, N], BF16, name=f"xT_{k}") for k in range(KC)]
    xq = [nc.sync, nc.scalar, nc.sync, nc.scalar]
    for k in range(KC):
        xq[k].dma_start(pre["xT"][k][:], d["xT"][k * 128 : (k + 1) * 128, :])
    pre["W1"] = [pool.tile([128, 3 * DIM], BF16, name=f"W1_{k}") for k in range(KC)]
    pre["W2"] = [pool.tile([128, 3 * DIM], BF16, name=f"W2_{k}") for k in range(KC)]
    for blk in range(3):  # q, k, v column blocks — earliest-needed first
        for k in range(KC):
            nc.sync.dma_start(
                pre["W1"][k][:, blk * DIM : (blk + 1) * DIM],
                d["Wqkv1"][k * 128 : (k + 1) * 128, blk * DIM : (blk + 1) * DIM],
            )
    # mask, duplicated per ic host-side: [j, 2048] = [ic0|ic0|ic1|ic1]
    pre["maskTd"] = [pool.tile([128, 2 * N], BF16, name=f"maskTd_{j}") for j in range(JC)]
    for j in range(JC):
        q = nc.scalar if j % 2 == 0 else nc.sync
        q.dma_start(pre["maskTd"][j][:], d["maskTd"][j * 128 : (j + 1) * 128, :])
    for blk in range(3):
        for k in range(KC):
            nc.scalar.dma_start(
                pre["W2"][k][:, blk * DIM : (blk + 1) * DIM],
                d["Wqkv2"][k * 128 : (k + 1) * 128, blk * DIM : (blk + 1) * DIM],
            )
    pre["Wnn"] = [pool.tile([128, DIM], BF16, name=f"Wnn_{k}") for k in range(KC)]
    for k in range(KC):
        nc.sync.dma_start(pre["Wnn"][k][:], d["Wnn1"][k * 128 : (k + 1) * 128, :])

    if not zb:
        for nm, srcn, off in (("bq1", "bqkv1", 0), ("bk1", "bqkv1", DIM),
                              ("bq2", "bqkv2", 0), ("bk2", "bqkv2", DIM)):
            tiles = [pool.tile([128, 1], F32, name=f"{nm}_{t}") for t in range(4)]
            for t in range(4):
                nc.sync.dma_start(
                    tiles[t][:], d[srcn][off + t * 128 : off + (t + 1) * 128]
                )
            pre[nm] = tiles
        pre["brow"] = [pool.tile([1, DIM], BF16, name=f"brow_{r}") for r in range(3)]
        for r in range(3):
            nc.sync.dma_start(pre["brow"][r][:], d["brows"][r : r + 1, :])
        pre["ones1"] = pool.tile([1, 128], BF16, name="ones1")
        nc.vector.memset(pre["ones1"][:], 1.0)
    else:
        pre["bq1"] = pre["bk1"] = pre["bq2"] = pre["bk2"] = [None] * 4
        pre["brow"] = [None] * 3
        pre["ones1"] = None
    pre["expb"] = pool.tile([128, 1], F32, name="expb")
    nc.vector.memset(pre["expb"][:], EXP_BIAS)
    # warm the sigmoid ACT table before the first body needs it
    warm = pool.tile([1, 1], F32, name="warm")
    nc.vector.memset(warm[:], 0.0)
    nc.scalar.activation(warm[:], warm[:], AF.Sigmoid)
    pre["warm"] = warm
    return pre


def build_body(ctx: ExitStack, tc: tile.TileContext, d, out_ap, zb=False,
               pre=None):
    nc = tc.nc

    bq1, bk1, bq2, bk2 = pre["bq1"], pre["bk1"], pre["bq2"], pre["bk2"]
    brow, ones1, expb = pre["brow"], pre["ones1"], pre["expb"]

    # --- tensors that span stage boundaries -------------------------------
    o1_pool = ctx.enter_context(tc.tile_pool(name="o1", bufs=1))
    O1T = [o1_pool.tile([128, N], BF16, name=f"O1T_{t}") for t in range(4)]

    s1 = ctx.enter_context(ExitStack())  # stage-1 scope: closed after phase 2
    qk1_pool = s1.enter_context(tc.tile_pool(name="qk1", bufs=1))
    QT1 = [qk1_pool.tile([128, N], BF16, name=f"QT1_{t}") for t in range(4)]
    KT1 = [qk1_pool.tile([128, N], BF16, name=f"KT1_{t}") for t in range(4)]
    V1 = [qk1_pool.tile([128, DIM], BF16, name=f"V1_{j}") for j in range(JC)]

    maskTd = pre["maskTd"]

    def proj_qk(nc, pool_ps, w_sb, bias_sb, src_sb, dst, col0, pfx, pairs,
                evict="act"):
        """dst[c, i] for weight cols [col0+t*128, ..): dst = W.T @ src + b."""
        for t in pairs:
            for ic in range(IC):
                ps = pool_ps.tile([128, 512], F32, tag="proj_ps", name=f"{pfx}_{t}_{ic}")
                for kc in range(KC):
                    nc.tensor.matmul(
                        ps[:],
                        w_sb[kc][:, col0 + t * 128 : col0 + (t + 1) * 128],
                        src_sb[kc][:, ic * 512 : (ic + 1) * 512],
                        start=(kc == 0),
                        stop=(kc == KC - 1),
                    )
                dslc = dst[t][:, ic * 512 : (ic + 1) * 512]
                if zb:
                    if evict == "act":
                        nc.scalar.copy(dslc, ps[:])
                    else:
                        nc.vector.tensor_copy(dslc, ps[:])
                elif evict == "act":
                    nc.scalar.activation(
                        dslc, ps[:], AF.Identity, bias=bias_sb[t][:]
                    )
                else:
                    nc.vector.tensor_scalar_add(dslc, ps[:], bias_sb[t][:])

    # =====================================================================
    # Phase 1: stage-1 projections
    # =====================================================================
    with tc.tile_pool(name="ps1", bufs=4, space="PSUM") as ps1_pool:
        xT = pre["xT"]
        W1 = pre["W1"]

        # head-pair-0 Q/K first so pair-0 scores can start ASAP, then V
        # (pair-0 apply needs it), then the remaining pairs.
        proj_qk(nc, ps1_pool, W1, bq1, xT, QT1, 0, "q1", pairs=(0,))
        proj_qk(nc, ps1_pool, W1, bk1, xT, KT1, DIM, "k1", pairs=(0,))
        for j in range(JC):
            ps = ps1_pool.tile([128, 512], F32, tag="proj_ps", name=f"vps_{j}")
            if not zb:
                nc.tensor.matmul(ps[:], ones1[:], brow[0][:], start=True, stop=False)
            for kc in range(KC):
                nc.tensor.matmul(
                    ps[:],
                    xT[kc][:, j * 128 : (j + 1) * 128],
                    W1[kc][:, 2 * DIM : 3 * DIM],
                    start=(zb and kc == 0),
                    stop=(kc == KC - 1),
                )
            nc.scalar.copy(V1[j][:], ps[:])
        for t in range(1, 4):
            proj_qk(nc, ps1_pool, W1, bq1, xT, QT1, 0, "q1", pairs=(t,))
            proj_qk(nc, ps1_pool, W1, bk1, xT, KT1, DIM, "k1", pairs=(t,))

    if _STOP_PHASE <= 1:
        raise StopIteration

    # =====================================================================
    # Phase 2: stage-1 attention (sigmoid(S * mask) @ V), transposed
    #   P1 layout per pair: [128, 8192], block(j, ic, hh) at
    #   j*2048 + ic*1024 + hh*512
    # =====================================================================
    with tc.tile_pool(name="p1", bufs=4) as p_pool, \
         tc.tile_pool(name="ptmp", bufs=2) as ptmp_pool, \
         tc.tile_pool(name="sps1", bufs=2, space="PSUM") as score_ps, \
         tc.tile_pool(name="aps1", bufs=4, space="PSUM") as apply_ps:

        def emit_apply1(t, P1q, aps, j):
            # col-tiled: h_even -> rows 0:64, h_odd -> rows 64:128
            off = (j % 2) * 2048
            for ic in range(IC):
                for hh in range(2):
                    nc.tensor.matmul(
                        aps[ic][hh * 64 : (hh + 1) * 64, :],
                        V1[j][:, (2 * t + hh) * D : (2 * t + hh + 1) * D],
                        P1q[j // 2][:, off + ic * 1024 + hh * 512 :
                                    off + ic * 1024 + hh * 512 + 512],
                        start=(j == 0),
                        stop=(j == JC - 1),
                    )

        def evict_apply1(t, aps):
            for ic in range(IC):
                nc.scalar.copy(
                    O1T[t][:, ic * 512 : (ic + 1) * 512], aps[ic][:]
                )

        # uniform lag-2 pipeline: apply(t, j-2) inside pair t's own loop,
        # P1 in [128, 4096] quarter tiles (one per sigmoid block) so the
        # applies chase the sigmoids without whole-pair serialization.
        for t in range(4):
            P1q = [
                p_pool.tile([128, 4096], BF16, tag="p1", name=f"P1_{t}_{g}")
                for g in range(4)
            ]
            aps = [
                apply_ps.tile([128, 512], F32, tag="aps1", name=f"aps1_{t}_{i}")
                for i in range(IC)
            ]
            pt_sb = None
            for j in range(JC):
                if j % 2 == 0:  # staging for a 2-j sigmoid block
                    pt_sb = ptmp_pool.tile(
                        [128, 4096], BF16, tag="pt", name=f"pt_{t}_{j}"
                    )
                S = [
                    score_ps.tile([128, 1024], F32, tag="s1", name=f"s1_{t}_{j}_{ic}")
                    for ic in range(IC)
                ]
                # 4 score MMs, adjacent, alternating row groups (h0/h64)
                for ic in range(IC):
                    for hh in range(2):
                        base = 64 * hh
                        nc.tensor.matmul(
                            S[ic][:, hh * 512 : (hh + 1) * 512],
                            KT1[t][base : base + 64, j * 128 : (j + 1) * 128],
                            QT1[t][base : base + 64, ic * 512 : (ic + 1) * 512],
                            start=True,
                            stop=True,
                        )
                for ic in range(IC):
                    nc.vector.tensor_tensor(
                        pt_sb[:, (j % 2) * 2048 + ic * 1024 :
                              (j % 2) * 2048 + (ic + 1) * 1024],
                        S[ic][:],
                        maskTd[j][:, ic * 1024 : (ic + 1) * 1024],
                        ALU.mult,
                    )
                if j % 2 == 1:  # two j-blocks complete -> one [128,4096] sigmoid
                    nc.scalar.activation(
                        P1q[j // 2][:],
                        pt_sb[:],
                        AF.Sigmoid,
                    )
                if j >= 2:
                    emit_apply1(t, P1q, aps, j - 2)
            emit_apply1(t, P1q, aps, JC - 2)
            emit_apply1(t, P1q, aps, JC - 1)
            evict_apply1(t, aps)

    # prewarm the exp table in the stage-1 -> stage-2 transition gap
    nc.scalar.activation(pre["warm"][:], pre["warm"][:], AF.Exp)

    if _STOP_PHASE <= 2:
        raise StopIteration
    s1.close()  # free QT1/KT1/V1

    # =====================================================================
    # Phase 3: stage-2 projections (from O1T)
    # =====================================================================
    qk2_pool = ctx.enter_context(tc.tile_pool(name="qk2", bufs=1))
    QT2 = [qk2_pool.tile([128, N], BF16, name=f"QT2_{t}") for t in range(4)]
    KT2 = [qk2_pool.tile([128, N], BF16, name=f"KT2_{t}") for t in range(4)]
    V2p = [qk2_pool.tile([128, H * VP], BF16, name=f"V2p_{j}") for j in range(JC)]

    # phase-4 pools open BEFORE ps2 so the score pool gets PSUM banks
    # disjoint from the projection pool.
    s4 = ExitStack()
    o2_pool = ctx.enter_context(tc.tile_pool(name="o2", bufs=1))
    O2T = [o2_pool.tile([128, N], BF16, name=f"O2T_{t}") for t in range(4)]
    p2_pool = s4.enter_context(tc.tile_pool(name="p2", bufs=4))
    d_pool = s4.enter_context(tc.tile_pool(name="dscr", bufs=4))
    score2_ps = s4.enter_context(tc.tile_pool(name="sps2", bufs=2, space="PSUM"))

    def alloc_P2(t):
        # quarter tiles: block(j, ic, hh) at P2q[j//2][(j%2)*2048+ic*1024+hh*512]
        return [
            p2_pool.tile([128, 4096], BF16, tag="p2", name=f"P2_{t}_{g}")
            for g in range(4)
        ]

    def emit_scores2_j(t, P2q, j):
        S = [
            score2_ps.tile([128, 1024], F32, tag="s2", name=f"s2_{t}_{j}_{ic}")
            for ic in range(IC)
        ]
        for ic in range(IC):
            for hh in range(2):
                base = 64 * hh
                nc.tensor.matmul(
                    S[ic][:, hh * 512 : (hh + 1) * 512],
                    KT2[t][base : base + 64, j * 128 : (j + 1) * 128],
                    QT2[t][base : base + 64, ic * 512 : (ic + 1) * 512],
                    start=True,
                    stop=True,
                )
        off = (j % 2) * 2048
        for ic in range(IC):
            nc.scalar.activation(
                P2q[j // 2][:, off + ic * 1024 : off + (ic + 1) * 1024],
                S[ic][:],
                AF.Exp,
                bias=expb[:],
                scale=SCALE,
            )

    with tc.tile_pool(name="ps2", bufs=4, space="PSUM") as ps2_pool:
        W2 = pre["W2"]

        proj_qk(nc, ps2_pool, W2, bq2, O1T, QT2, 0, "q2", pairs=(0,), evict="dve")
        proj_qk(nc, ps2_pool, W2, bk2, O1T, KT2, DIM, "k2", pairs=(0,), evict="dve")
        # hoisted: pair-0 stage-2 scores+exp overlap the remaining projections
        P2q0 = alloc_P2(0)
        for j in range(JC):
            emit_scores2_j(0, P2q0, j)
        for t in range(1, 4):
            proj_qk(nc, ps2_pool, W2, bq2, O1T, QT2, 0, "q2", pairs=(t,), evict="dve")
            proj_qk(nc, ps2_pool, W2, bk2, O1T, KT2, DIM, "k2", pairs=(t,), evict="dve")
        for j in range(JC):
            ps = ps2_pool.tile([128, 512], F32, tag="proj_ps", name=f"v2ps_{j}")
            if not zb:
                nc.tensor.matmul(ps[:], ones1[:], brow[1][:], start=True, stop=False)
            for kc in range(KC):
                nc.tensor.matmul(
                    ps[:],
                    O1T[kc][:, j * 128 : (j + 1) * 128],
                    W2[kc][:, 2 * DIM : 3 * DIM],
                    start=(zb and kc == 0),
                    stop=(kc == KC - 1),
                )
            # scatter per-head into the padded layout [j, h*128 + d]
            nc.vector.tensor_copy(
                V2p[j][:, :].rearrange("p (h e) -> p h e", e=VP)[:, :, :D],
                ps[:].rearrange("p (h dd) -> p h dd", dd=D),
            )
            # 64 ones columns per head (drives matmul-replicated denominators)
            nc.vector.memset(
                V2p[j][:, :].rearrange("p (h e) -> p h e", e=VP)[:, :, D:VP], 1.0
            )

    if _STOP_PHASE <= 3:
        s4.close()
        raise StopIteration

    # =====================================================================
    # Phase 4: stage-2 attention (softmax via exp + replicated denominators)
    # =====================================================================
    apply2_ps = s4.enter_context(tc.tile_pool(name="aps2", bufs=4, space="PSUM"))

    def emit_apply2(t, P2q, aps, j):
        # ones-padded stationary: PSUM rows 0:64 = unnormalized out,
        # rows 64:128 = softmax denominator replicated 64x
        off = (j % 2) * 2048
        for ic in range(IC):
            for hh in range(2):
                h = 2 * t + hh
                nc.tensor.matmul(
                    aps[hh][ic][:, :],
                    V2p[j][:, h * VP : (h + 1) * VP],
                    P2q[j // 2][:, off + ic * 1024 + hh * 512 :
                                off + ic * 1024 + hh * 512 + 512],
                    start=(j == 0),
                    stop=(j == JC - 1),
                )

    def evict_apply2(t, aps):
        # ic0 first so next-pair applies / phase 5 unblock earliest
        for ic in range(IC):
            for hh in range(2):
                ds = d_pool.tile([64, 512], F32, tag="ds", name=f"ds_{hh}_{ic}")
                db = d_pool.tile([64, 512], F32, tag="db", name=f"db_{hh}_{ic}")
                nc.vector.tensor_copy(ds[:], aps[hh][ic][64:128, :])
                nc.vector.reciprocal_approx_fast(db[:], ds[:])
                nc.vector.tensor_tensor(
                    O2T[t][hh * 64 : (hh + 1) * 64, ic * 512 : (ic + 1) * 512],
                    aps[hh][ic][0:64, :],
                    db[:],
                    ALU.mult,
                )

    def alloc_aps2(t):
        return [
            [
                apply2_ps.tile([128, 512], F32, tag="aps2", name=f"aps2_{t}_{hh}_{i}")
                for i in range(IC)
            ]
            for hh in range(2)
        ]

    # uniform lag-2 pipeline (pair-0 scores+exp already hoisted to phase 3)
    for t in range(4):
        aps = alloc_aps2(t)
        if t == 0:
            for j in range(JC):
                emit_apply2(0, P2q0, aps, j)
        else:
            P2q = alloc_P2(t)
            for j in range(JC):
                emit_scores2_j(t, P2q, j)
                if j >= 2:
                    emit_apply2(t, P2q, aps, j - 2)
            emit_apply2(t, P2q, aps, JC - 2)
            emit_apply2(t, P2q, aps, JC - 1)
        evict_apply2(t, aps)
    # swap the ACT table back to sigmoid for the next body while phase 5 runs
    nc.scalar.activation(pre["warm"][:], pre["warm"][:], AF.Sigmoid)
    s4.close()

    if _STOP_PHASE <= 4:
        raise StopIteration

    # =====================================================================
    # Phase 5: output projection
    # =====================================================================
    with tc.tile_pool(name="outst", bufs=6) as out_pool, \
         tc.tile_pool(name="ps5", bufs=4, space="PSUM") as ps5_pool:
        Wnn = pre["Wnn"]
        oq = [nc.sync, nc.scalar]
        for i8 in range(JC):  # 8 chunks of 128 output rows
            ps = ps5_pool.tile([128, 512], F32, tag="out_ps", name=f"ops_{i8}")
            if not zb:
                nc.tensor.matmul(ps[:], ones1[:], brow[2][:], start=True, stop=False)
            for kc in range(KC):
                nc.tensor.matmul(
                    ps[:],
                    O2T[kc][:, i8 * 128 : (i8 + 1) * 128],
                    Wnn[kc][:],
                    start=(zb and kc == 0),
                    stop=(kc == KC - 1),
                )
            ob = out_pool.tile([128, DIM], F32, tag="ob", name=f"ob_{i8}")
            nc.scalar.copy(ob[:], ps[:])
            oq[i8 % 2].dma_start(out_ap[i8 * 128 : (i8 + 1) * 128, :], ob[:])


def build(n_repeat: int = 1, debug_taps: bool = False, n_loop: int = 0,
          zero_bias: bool = False):
    install_patches()
    nc = bacc.Bacc("TRN2", target_bir_lowering=False, debug=False)
    d = {}

    def din(name, shape, dtype=F32):
        d[name] = nc.dram_tensor(name, shape, dtype, kind="ExternalInput").ap()

    din("xT", [DIM, N], BF16)
    din("maskTd", [N, 2 * N], BF16)
    din("Wqkv1", [DIM, 3 * DIM], BF16)
    din("Wqkv2", [DIM, 3 * DIM], BF16)
    din("Wnn1", [DIM, DIM], BF16)
    if not zero_bias:
        din("bqkv1", [3 * DIM])
        din("bqkv2", [3 * DIM])
        din("brows", [3, DIM], BF16)
    out_ap = nc.dram_tensor("out", [N, DIM], F32, kind="ExternalOutput").ap()

    with tile.TileContext(nc) as tc:
        with ExitStack() as pctx:
            pre = build_prologue(pctx, tc, d, zb=zero_bias)
            if n_loop:
                with tc.For_i(0, n_loop):
                    with ExitStack() as ctx:
                        try:
                            build_body(ctx, tc, d, out_ap, zb=zero_bias, pre=pre)
                        except StopIteration:
                            pass
            else:
                for _ in range(n_repeat):
                    with ExitStack() as ctx:
                        try:
                            build_body(ctx, tc, d, out_ap, zb=zero_bias, pre=pre)
                        except StopIteration:
                            pass

    nc.compile()
    n = legalize_single_wait(nc)
    return nc, n


# ===========================================================================
# Host-side entry point: full inputs in, full output out.
# Sharding: pure data-parallel — B=8 batch elements, one per NeuronCore.
# ===========================================================================
import numpy as np

_CACHED = {}


def _get_program(zero_bias=False):
    key = ("nc", zero_bias)
    if key not in _CACHED:
        _CACHED[key] = build(n_repeat=1, zero_bias=zero_bias)[0]
    return _CACHED[key]


def _bf16(a):
    return np.asarray(a, dtype=mybir.dt.np(BF16))


def _make_common(mask, Wqkv1, bqkv1, Wqkv2, bqkv2, Wnn1, bnn1,
                 zero_bias=False):
    f32 = lambda a: np.ascontiguousarray(np.asarray(a, dtype=np.float32))
    bqkv1, bqkv2, bnn1 = f32(bqkv1), f32(bqkv2), f32(bnn1)
    mT = np.asarray(mask, np.float32)[0, 0].T  # [keys, queries]
    maskTd = np.concatenate(
        [mT[:, :512], mT[:, :512], mT[:, 512:], mT[:, 512:]], axis=1
    )
    common = {
        "maskTd": np.ascontiguousarray(_bf16(maskTd)),
        "Wqkv1": np.ascontiguousarray(_bf16(Wqkv1)),
        "Wqkv2": np.ascontiguousarray(_bf16(Wqkv2)),
        "Wnn1": np.ascontiguousarray(_bf16(Wnn1)),
    }
    if not zero_bias:
        brows = np.stack([bqkv1[2 * DIM :], bqkv2[2 * DIM :], bnn1])
        common.update({
            "bqkv1": bqkv1,
            "bqkv2": bqkv2,
            "brows": np.ascontiguousarray(_bf16(brows)),
        })
    return common


def _bias_free(*biases):
    return all(not np.any(np.asarray(b)) for b in biases)


def kernel(x, mask, Wqkv1, bqkv1, Wqkv2, bqkv2, Wnn1, bnn1):
    from concourse.bass_utils import run_bass_kernel_spmd

    x = np.asarray(x, dtype=np.float32)
    zb = _bias_free(bqkv1, bqkv2, bnn1)
    common = _make_common(mask, Wqkv1, bqkv1, Wqkv2, bqkv2, Wnn1, bnn1,
                          zero_bias=zb)
    in_maps = [
        {"xT": np.ascontiguousarray(_bf16(x[c].T)), **common}
        for c in range(x.shape[0])
    ]
    nc = _get_program(zero_bias=zb)
    res = run_bass_kernel_spmd(nc, in_maps, core_ids=list(range(8)))
    return np.stack([res.results[c]["out"] for c in range(8)]).astype(np.float32)


# revision 16
# speedup vs baseline: 1.2909x; 1.2909x over previous
"""Bass/Tile kernel for the two-stage attention block (v4).

Layout (from v3): everything on-chip is transposed ([feature, token],
feature on partitions) so both attention stages chain with zero on-chip
transposes:

  QT/KT  = W.T @ x.T       : matmul(lhsT=W_chunk, rhs=xT_chunk)   -> [c, i]
  V      = x @ W           : matmul(lhsT=xT_chunk, rhs=Wv_chunk)  -> [j, c]
  S^T    = (q@k.T).T       : matmul(lhsT=KT_h, rhs=QT_h)          -> [j, i]
  P^T    = act(S^T * m^T)  : elementwise
  O^T    = (P@v).T         : matmul(lhsT=V_h, rhs=P^T_h)          -> [d, i]

v4 changes (driven by NTFF profiling: HAM clock-gate thrash + DVE
reciprocal chains + no PE-tile concurrency):
- Score matmuls for the two heads of a pair write one combined PSUM tile
  ([h_even 512 | h_odd 512]) and are emitted adjacently with alternating
  row groups (lhsT base partition 0/64) -> the PE array runs them
  concurrently (row tiling, ~2x measured).
- Stage-1 apply matmuls are col-tiled: h_even -> PSUM rows 0:64,
  h_odd -> rows 64:128 of the same tile (~2x), which also makes the
  eviction a single [128,512] copy and double-buffers apply PSUM across
  head pairs (no pair-boundary stall).
- Softmax denominators: reciprocal_approx_fast (~5x faster than
  reciprocal; 18 bits, plenty for bf16 data) and evictions emitted
  ic0-first so phase 5 / next pair unblock early.  This removes the
  5-12us PE-idle windows at pair boundaries that re-engaged the HAM
  clock gate (PE at 1.2GHz instead of 2.4GHz for ~40% of the kernel).
- Mask is pre-duplicated host-side ([j, ic0|ic0|ic1|ic1] layout) so the
  stage-1 mask multiply stays one [128,1024] DVE op per (j, ic).
"""

from contextlib import ExitStack

import concourse.bacc as bacc
import concourse.bass as bass
import concourse.tile as tile
from concourse import mybir
from concourse.vector_clock import ScopedClock

F32 = mybir.dt.float32
BF16 = mybir.dt.bfloat16
AF = mybir.ActivationFunctionType
ALU = mybir.AluOpType

N, DIM, H, D = 1024, 512, 8, 64
SCALE = DIM**-0.5
KC = DIM // 128  # contraction chunks for projections
JC = N // 128  # key-side chunks (128 wide)
IC = N // 512  # query-side chunks for 512-wide matmul outputs
EXP_BIAS = -15.0
VP = 2 * D  # per-head width in padded V2: 64 data cols + 64 ones cols
_STOP_PHASE = 99


# ---------------------------------------------------------------------------
# Walrus in this container rejects instructions with >1 sync-wait.
# Split: hoist extra waits onto single-wait NoOps inserted just before.
def legalize_single_wait(nc):
    n_split = 0
    for fn in nc.m.functions:
        for blk in fn.blocks:
            insts = list(blk.instructions)
            out = []
            changed = False
            for inst in insts:
                si = inst.sync_info
                waits = list(si.on_wait) if (si is not None and si.on_wait) else []
                if len(waits) > 1:
                    changed = True
                    n_split += len(waits) - 1
                    for w in waits[:-1]:
                        nop = mybir.InstNoOp(
                            name=nc.get_next_instruction_name(),
                            sync_info=mybir.SyncInfo(on_wait=[w], on_update=[]),
                            bass_nofuse=True,
                            engine=inst.engine,
                        )
                        nc.register_instruction(nop)
                        out.append(nop)
                    si.on_wait = [waits[-1]]
                out.append(inst)
            if changed:
                blk.instructions = out
    return n_split


def _patched_drain_and_barrier(self, tick_clock, wait_clock):
    drain_inst = self.nc.sync.drain()
    wait_clock.add_sem_waits(
        drain_inst.ins, ScopedClock({None: tick_clock.global_clock})
    )
    si = drain_inst.ins.sync_info
    waits = list(si.on_wait or []) if si is not None else []
    if len(waits) > 1:
        si.on_wait = [waits[0]]
        for w in waits[1:]:
            extra = self.nc.sync.drain()
            esi = extra.ins.sync_info
            if esi is None:
                extra.ins.sync_info = mybir.SyncInfo(on_wait=[w], on_update=[])
            else:
                esi.on_wait = [w]

    self.nc.all_engine_barrier()
    assert self.sems is not None
    popped = self.nc._tile_sem_poison_stack.pop()
    assert popped is self._sem_poison
    self.nc.clear_and_free_semaphores(list(self.sems.allocated().values()))
    self.nc.all_engine_barrier()


def install_patches():
    tile.TileContext._drain_and_barrier = _patched_drain_and_barrier


# ---------------------------------------------------------------------------


def build_prologue(ctx: ExitStack, tc: tile.TileContext, d, zb=False):
    """Load loop-invariant tensors (inputs, weights, mask, consts) once."""
    nc = tc.nc
    pool = ctx.enter_context(tc.tile_pool(name="persist", bufs=1))
    pre = {}

    pre["xT"] = [pool.tile([128, N], BF16, name=f"xT_{k}") for k in range(KC)]
    xq = [nc.sync, nc.scalar, nc.sync, nc.scalar]
    for k in range(KC):
        xq[k].dma_start(pre["xT"][k][:], d["xT"][k * 128 : (k + 1) * 128, :])
    pre["W1"] = [pool.tile([128, 3 * DIM], BF16, name=f"W1_{k}") for k in range(KC)]
    pre["W2"] = [pool.tile([128, 3 * DIM], BF16, name=f"W2_{k}") for k in range(KC)]
    for blk in range(3):  # q, k, v column blocks — earliest-needed first
        for k in range(KC):
            nc.sync.dma_start(
                pre["W1"][k][:, blk * DIM : (blk + 1) * DIM],
                d["Wqkv1"][k * 128 : (k + 1) * 128, blk * DIM : (blk + 1) * DIM],
            )
    # mask, duplicated per ic host-side: [j, 2048] = [ic0|ic0|ic1|ic1]
    pre["maskTd"] = [pool.tile([128, 2 * N], BF16, name=f"maskTd_{j}") for j in range(JC)]
    for j in range(JC):
        q = nc.scalar if j % 2 == 0 else nc.sync
        q.dma_start(pre["maskTd"][j][:], d["maskTd"][j * 128 : (j + 1) * 128, :])
    for blk in range(3):
        for k in range(KC):
            nc.scalar.dma_start(
                pre["W2"][k][:, blk * DIM : (blk + 1) * DIM],
                d["Wqkv2"][k * 128 : (k + 1) * 128, blk * DIM : (blk + 1) * DIM],
            )
    pre["Wnn"] = [pool.tile([128, DIM], BF16, name=f"Wnn_{k}") for k in range(KC)]
    for k in range(KC):
        nc.sync.dma_start(pre["Wnn"][k][:], d["Wnn1"][k * 128 : (k + 1) * 128, :])

    if not zb:
        for nm, srcn, off in (("bq1", "bqkv1", 0), ("bk1", "bqkv1", DIM),
                              ("bq2", "bqkv2", 0), ("bk2", "bqkv2", DIM)):
            tiles = [pool.tile([128, 1], F32, name=f"{nm}_{t}") for t in range(4)]
            for t in range(4):
                nc.sync.dma_start(
                    tiles[t][:], d[srcn][off + t * 128 : off + (t + 1) * 128]
                )
            pre[nm] = tiles
        pre["brow"] = [pool.tile([1, DIM], BF16, name=f"brow_{r}") for r in range(3)]
        for r in range(3):
            nc.sync.dma_start(pre["brow"][r][:], d["brows"][r : r + 1, :])
        pre["ones1"] = pool.tile([1, 128], BF16, name="ones1")
        nc.vector.memset(pre["ones1"][:], 1.0)
    else:
        pre["bq1"] = pre["bk1"] = pre["bq2"] = pre["bk2"] = [None] * 4
        pre["brow"] = [None] * 3
        pre["ones1"] = None
    pre["expb"] = pool.tile([128, 1], F32, name="expb")
    nc.vector.memset(pre["expb"][:], EXP_BIAS)
    # warm the sigmoid ACT table before the first body needs it
    warm = pool.tile([1, 1], F32, name="warm")
    nc.vector.memset(warm[:], 0.0)
    nc.scalar.activation(warm[:], warm[:], AF.Sigmoid)
    pre["warm"] = warm
    return pre


def build_body(ctx: ExitStack, tc: tile.TileContext, d, out_ap, zb=False,
               pre=None):
    nc = tc.nc

    bq1, bk1, bq2, bk2 = pre["bq1"], pre["bk1"], pre["bq2"], pre["bk2"]
    brow, ones1, expb = pre["brow"], pre["ones1"], pre["expb"]

    # --- tensors that span stage boundaries -------------------------------
    o1_pool = ctx.enter_context(tc.tile_pool(name="o1", bufs=1))
    O1T = [o1_pool.tile([128, N], BF16, name=f"O1T_{t}") for t in range(4)]

    s1 = ctx.enter_context(ExitStack())  # stage-1 scope: closed after phase 2
    qk1_pool = s1.enter_context(tc.tile_pool(name="qk1", bufs=1))
    QT1 = [qk1_pool.tile([128, N], BF16, name=f"QT1_{t}") for t in range(4)]
    KT1 = [qk1_pool.tile([128, N], BF16, name=f"KT1_{t}") for t in range(4)]
    V1 = [qk1_pool.tile([128, DIM], BF16, name=f"V1_{j}") for j in range(JC)]

    maskTd = pre["maskTd"]

    def proj_qk(nc, pool_ps, w_sb, bias_sb, src_sb, dst, col0, pfx, pairs,
                evict="act"):
        """dst[c, i] for weight cols [col0+t*128, ..): dst = W.T @ src + b."""
        for t in pairs:
            for ic in range(IC):
                ps = pool_ps.tile([128, 512], F32, tag="proj_ps", name=f"{pfx}_{t}_{ic}")
                for kc in range(KC):
                    nc.tensor.matmul(
                        ps[:],
                        w_sb[kc][:, col0 + t * 128 : col0 + (t + 1) * 128],
                        src_sb[kc][:, ic * 512 : (ic + 1) * 512],
                        start=(kc == 0),
                        stop=(kc == KC - 1),
                    )
                dslc = dst[t][:, ic * 512 : (ic + 1) * 512]
                if zb:
                    if evict == "act":
                        nc.scalar.copy(dslc, ps[:])
                    else:
                        nc.vector.tensor_copy(dslc, ps[:])
                elif evict == "act":
                    nc.scalar.activation(
                        dslc, ps[:], AF.Identity, bias=bias_sb[t][:]
                    )
                else:
                    nc.vector.tensor_scalar_add(dslc, ps[:], bias_sb[t][:])

    # =====================================================================
    # Phase 1: stage-1 projections
    # =====================================================================
    with tc.tile_pool(name="ps1", bufs=4, space="PSUM") as ps1_pool:
        xT = pre["xT"]
        W1 = pre["W1"]

        # head-pair-0 Q/K first so pair-0 scores can start ASAP, then V
        # (pair-0 apply needs it), then the remaining pairs.
        proj_qk(nc, ps1_pool, W1, bq1, xT, QT1, 0, "q1", pairs=(0,))
        proj_qk(nc, ps1_pool, W1, bk1, xT, KT1, DIM, "k1", pairs=(0,))
        for j in range(JC):
            ps = ps1_pool.tile([128, 512], F32, tag="proj_ps", name=f"vps_{j}")
            if not zb:
                nc.tensor.matmul(ps[:], ones1[:], brow[0][:], start=True, stop=False)
            for kc in range(KC):
                nc.tensor.matmul(
                    ps[:],
                    xT[kc][:, j * 128 : (j + 1) * 128],
                    W1[kc][:, 2 * DIM : 3 * DIM],
                    start=(zb and kc == 0),
                    stop=(kc == KC - 1),
                )
            nc.scalar.copy(V1[j][:], ps[:])
        for t in range(1, 4):
            proj_qk(nc, ps1_pool, W1, bq1, xT, QT1, 0, "q1", pairs=(t,))
            proj_qk(nc, ps1_pool, W1, bk1, xT, KT1, DIM, "k1", pairs=(t,))

    if _STOP_PHASE <= 1:
        raise StopIteration

    # =====================================================================
    # Phase 2: stage-1 attention (sigmoid(S * mask) @ V), transposed
    #   P1 layout per pair: [128, 8192], block(j, ic, hh) at
    #   j*2048 + ic*1024 + hh*512
    # =====================================================================
    with tc.tile_pool(name="p1", bufs=4) as p_pool, \
         tc.tile_pool(name="ptmp", bufs=2) as ptmp_pool, \
         tc.tile_pool(name="sps1", bufs=2, space="PSUM") as score_ps, \
         tc.tile_pool(name="aps1", bufs=4, space="PSUM") as apply_ps:

        def emit_apply1(t, P1q, aps, j):
            # col-tiled: h_even -> rows 0:64, h_odd -> rows 64:128
            off = (j % 2) * 2048
            for ic in range(IC):
                for hh in range(2):
                    nc.tensor.matmul(
                        aps[ic][hh * 64 : (hh + 1) * 64, :],
                        V1[j][:, (2 * t + hh) * D : (2 * t + hh + 1) * D],
                        P1q[j // 2][:, off + ic * 1024 + hh * 512 :
                                    off + ic * 1024 + hh * 512 + 512],
                        start=(j == 0),
                        stop=(j == JC - 1),
                    )

        def evict_apply1(t, aps):
            for ic in range(IC):
                nc.scalar.copy(
                    O1T[t][:, ic * 512 : (ic + 1) * 512], aps[ic][:]
                )

        # prev-pair pipeline (dense tensor stream) with quarter P1 tiles so
        # the final pair's applies gate per-quarter, not per-pair.
        prev = None  # (pair_idx, P1q, aps)
        for t in range(4):
            P1q = [
                p_pool.tile([128, 4096], BF16, tag="p1", name=f"P1_{t}_{g}")
                for g in range(4)
            ]
            pt_sb = None
            for j in range(JC):
                if j % 2 == 0:  # staging for a 2-j sigmoid block
                    pt_sb = ptmp_pool.tile(
                        [128, 4096], BF16, tag="pt", name=f"pt_{t}_{j}"
                    )
                S = [
                    score_ps.tile([128, 1024], F32, tag="s1", name=f"s1_{t}_{j}_{ic}")
                    for ic in range(IC)
                ]
                # 4 score MMs, adjacent, alternating row groups (h0/h64)
                for ic in range(IC):
                    for hh in range(2):
                        base = 64 * hh
                        nc.tensor.matmul(
                            S[ic][:, hh * 512 : (hh + 1) * 512],
                            KT1[t][base : base + 64, j * 128 : (j + 1) * 128],
                            QT1[t][base : base + 64, ic * 512 : (ic + 1) * 512],
                            start=True,
                            stop=True,
                        )
                for ic in range(IC):
                    nc.vector.tensor_tensor(
                        pt_sb[:, (j % 2) * 2048 + ic * 1024 :
                              (j % 2) * 2048 + (ic + 1) * 1024],
                        S[ic][:],
                        maskTd[j][:, ic * 1024 : (ic + 1) * 1024],
                        ALU.mult,
                    )
                if j % 2 == 1:  # two j-blocks complete -> one [128,4096] sigmoid
                    nc.scalar.activation(
                        P1q[j // 2][:],
                        pt_sb[:],
                        AF.Sigmoid,
                    )
                if prev is not None:
                    emit_apply1(prev[0], prev[1], prev[2], j)
            if prev is not None:
                evict_apply1(prev[0], prev[2])
            aps = [
                apply_ps.tile([128, 512], F32, tag="aps1", name=f"aps1_{t}_{i}")
                for i in range(IC)
            ]
            prev = (t, P1q, aps)
        for j in range(JC):
            emit_apply1(prev[0], prev[1], prev[2], j)
        evict_apply1(prev[0], prev[2])

    # prewarm the exp table in the stage-1 -> stage-2 transition gap
    nc.scalar.activation(pre["warm"][:], pre["warm"][:], AF.Exp)

    if _STOP_PHASE <= 2:
        raise StopIteration
    s1.close()  # free QT1/KT1/V1

    # =====================================================================
    # Phase 3: stage-2 projections (from O1T)
    # =====================================================================
    qk2_pool = ctx.enter_context(tc.tile_pool(name="qk2", bufs=1))
    QT2 = [qk2_pool.tile([128, N], BF16, name=f"QT2_{t}") for t in range(4)]
    KT2 = [qk2_pool.tile([128, N], BF16, name=f"KT2_{t}") for t in range(4)]
    V2p = [qk2_pool.tile([128, H * VP], BF16, name=f"V2p_{j}") for j in range(JC)]

    # phase-4 pools open BEFORE ps2 so the score pool gets PSUM banks
    # disjoint from the projection pool.
    s4 = ExitStack()
    o2_pool = ctx.enter_context(tc.tile_pool(name="o2", bufs=1))
    O2T = [o2_pool.tile([128, N], BF16, name=f"O2T_{t}") for t in range(4)]
    p2_pool = s4.enter_context(tc.tile_pool(name="p2", bufs=4))
    d_pool = s4.enter_context(tc.tile_pool(name="dscr", bufs=4))
    score2_ps = s4.enter_context(tc.tile_pool(name="sps2", bufs=2, space="PSUM"))

    def alloc_P2(t):
        # quarter tiles: block(j, ic, hh) at P2q[j//2][(j%2)*2048+ic*1024+hh*512]
        return [
            p2_pool.tile([128, 4096], BF16, tag="p2", name=f"P2_{t}_{g}")
            for g in range(4)
        ]

    def emit_scores2_j(t, P2q, j):
        S = [
            score2_ps.tile([128, 1024], F32, tag="s2", name=f"s2_{t}_{j}_{ic}")
            for ic in range(IC)
        ]
        for ic in range(IC):
            for hh in range(2):
                base = 64 * hh
                nc.tensor.matmul(
                    S[ic][:, hh * 512 : (hh + 1) * 512],
                    KT2[t][base : base + 64, j * 128 : (j + 1) * 128],
                    QT2[t][base : base + 64, ic * 512 : (ic + 1) * 512],
                    start=True,
                    stop=True,
                )
        off = (j % 2) * 2048
        for ic in range(IC):
            nc.scalar.activation(
                P2q[j // 2][:, off + ic * 1024 : off + (ic + 1) * 1024],
                S[ic][:],
                AF.Exp,
                bias=expb[:],
                scale=SCALE,
            )

    with tc.tile_pool(name="ps2", bufs=4, space="PSUM") as ps2_pool:
        W2 = pre["W2"]

        proj_qk(nc, ps2_pool, W2, bq2, O1T, QT2, 0, "q2", pairs=(0,), evict="dve")
        proj_qk(nc, ps2_pool, W2, bk2, O1T, KT2, DIM, "k2", pairs=(0,), evict="dve")
        # hoisted: pair-0 stage-2 scores+exp overlap the remaining projections
        P2q0 = alloc_P2(0)
        for j in range(JC):
            emit_scores2_j(0, P2q0, j)
        for t in range(1, 4):
            proj_qk(nc, ps2_pool, W2, bq2, O1T, QT2, 0, "q2", pairs=(t,), evict="dve")
            proj_qk(nc, ps2_pool, W2, bk2, O1T, KT2, DIM, "k2", pairs=(t,), evict="dve")
        for j in range(JC):
            ps = ps2_pool.tile([128, 512], F32, tag="proj_ps", name=f"v2ps_{j}")
            if not zb:
                nc.tensor.matmul(ps[:], ones1[:], brow[1][:], start=True, stop=False)
            for kc in range(KC):
                nc.tensor.matmul(
                    ps[:],
                    O1T[kc][:, j * 128 : (j + 1) * 128],
                    W2[kc][:, 2 * DIM : 3 * DIM],
                    start=(zb and kc == 0),
                    stop=(kc == KC - 1),
                )
            # scatter per-head into the padded layout [j, h*128 + d]
            nc.vector.tensor_copy(
                V2p[j][:, :].rearrange("p (h e) -> p h e", e=VP)[:, :, :D],
                ps[:].rearrange("p (h dd) -> p h dd", dd=D),
            )
            # 64 ones columns per head (drives matmul-replicated denominators)
            nc.vector.memset(
                V2p[j][:, :].rearrange("p (h e) -> p h e", e=VP)[:, :, D:VP], 1.0
            )

    if _STOP_PHASE <= 3:
        s4.close()
        raise StopIteration

    # =====================================================================
    # Phase 4: stage-2 attention (softmax via exp + replicated denominators)
    # =====================================================================
    apply2_ps = s4.enter_context(tc.tile_pool(name="aps2", bufs=4, space="PSUM"))

    def emit_apply2(t, P2q, aps, j):
        # ones-padded stationary: PSUM rows 0:64 = unnormalized out,
        # rows 64:128 = softmax denominator replicated 64x
        off = (j % 2) * 2048
        for ic in range(IC):
            for hh in range(2):
                h = 2 * t + hh
                nc.tensor.matmul(
                    aps[hh][ic][:, :],
                    V2p[j][:, h * VP : (h + 1) * VP],
                    P2q[j // 2][:, off + ic * 1024 + hh * 512 :
                                off + ic * 1024 + hh * 512 + 512],
                    start=(j == 0),
                    stop=(j == JC - 1),
                )

    def evict_apply2(t, aps):
        # ic0 first so next-pair applies / phase 5 unblock earliest
        for ic in range(IC):
            for hh in range(2):
                ds = d_pool.tile([64, 512], F32, tag="ds", name=f"ds_{hh}_{ic}")
                db = d_pool.tile([64, 512], F32, tag="db", name=f"db_{hh}_{ic}")
                nc.vector.tensor_copy(ds[:], aps[hh][ic][64:128, :])
                nc.vector.reciprocal_approx_fast(db[:], ds[:])
                nc.vector.tensor_tensor(
                    O2T[t][hh * 64 : (hh + 1) * 64, ic * 512 : (ic + 1) * 512],
                    aps[hh][ic][0:64, :],
                    db[:],
                    ALU.mult,
                )

    def alloc_aps2(t):
        return [
            [
                apply2_ps.tile([128, 512], F32, tag="aps2", name=f"aps2_{t}_{hh}_{i}")
                for i in range(IC)
            ]
            for hh in range(2)
        ]

    # prev-pair pipeline with quarter P2 tiles (pair-0 scores+exp hoisted
    # to phase 3); the epilogue applies gate per-quarter.
    prev = (0, P2q0, alloc_aps2(0))
    for t in range(1, 4):
        P2q = alloc_P2(t)
        for j in range(JC):
            emit_scores2_j(t, P2q, j)
            emit_apply2(prev[0], prev[1], prev[2], j)
        evict_apply2(prev[0], prev[2])
        prev = (t, P2q, alloc_aps2(t))
    for j in range(JC):
        emit_apply2(prev[0], prev[1], prev[2], j)
    evict_apply2(prev[0], prev[2])
    # swap the ACT table back to sigmoid for the next body while phase 5 runs
    nc.scalar.activation(pre["warm"][:], pre["warm"][:], AF.Sigmoid)
    s4.close()

    if _STOP_PHASE <= 4:
        raise StopIteration

    # =====================================================================
    # Phase 5: output projection
    # =====================================================================
    with tc.tile_pool(name="outst", bufs=6) as out_pool, \
         tc.tile_pool(name="ps5", bufs=4, space="PSUM") as ps5_pool:
        Wnn = pre["Wnn"]
        oq = [nc.sync, nc.scalar]
        for i8 in range(JC):  # 8 chunks of 128 output rows
            ps = ps5_pool.tile([128, 512], F32, tag="out_ps", name=f"ops_{i8}")
            if not zb:
                nc.tensor.matmul(ps[:], ones1[:], brow[2][:], start=True, stop=False)
            for kc in range(KC):
                nc.tensor.matmul(
                    ps[:],
                    O2T[kc][:, i8 * 128 : (i8 + 1) * 128],
                    Wnn[kc][:],
                    start=(zb and kc == 0),
                    stop=(kc == KC - 1),
                )
            ob = out_pool.tile([128, DIM], F32, tag="ob", name=f"ob_{i8}")
            nc.scalar.copy(ob[:], ps[:])
            oq[i8 % 2].dma_start(out_ap[i8 * 128 : (i8 + 1) * 128, :], ob[:])


def build(n_repeat: int = 1, debug_taps: bool = False, n_loop: int = 0,
          zero_bias: bool = False):
    install_patches()
    nc = bacc.Bacc("TRN2", target_bir_lowering=False, debug=False)
    d = {}

    def din(name, shape, dtype=F32):
        d[name] = nc.dram_tensor(name, shape, dtype, kind="ExternalInput").ap()

    din("xT", [DIM, N], BF16)
    din("maskTd", [N, 2 * N], BF16)
    din("Wqkv1", [DIM, 3 * DIM], BF16)
    din("Wqkv2", [DIM, 3 * DIM], BF16)
    din("Wnn1", [DIM, DIM], BF16)
    if not zero_bias:
        din("bqkv1", [3 * DIM])
        din("bqkv2", [3 * DIM])
        din("brows", [3, DIM], BF16)
    out_ap = nc.dram_tensor("out", [N, DIM], F32, kind="ExternalOutput").ap()

    with tile.TileContext(nc) as tc:
        with ExitStack() as pctx:
            pre = build_prologue(pctx, tc, d, zb=zero_bias)
            if n_loop:
                with tc.For_i(0, n_loop):
                    with ExitStack() as ctx:
                        try:
                            build_body(ctx, tc, d, out_ap, zb=zero_bias, pre=pre)
                        except StopIteration:
                            pass
            else:
                for _ in range(n_repeat):
                    with ExitStack() as ctx:
                        try:
                            build_body(ctx, tc, d, out_ap, zb=zero_bias, pre=pre)
                        except StopIteration:
                            pass

    nc.compile()
    n = legalize_single_wait(nc)
    return nc, n


# ===========================================================================
# Host-side entry point: full inputs in, full output out.
# Sharding: pure data-parallel — B=8 batch elements, one per NeuronCore.
# ===========================================================================
import numpy as np

_CACHED = {}


def _get_program(zero_bias=False):
    key = ("nc", zero_bias)
    if key not in _CACHED:
        _CACHED[key] = build(n_repeat=1, zero_bias=zero_bias)[0]
    return _CACHED[key]


def _bf16(a):
    return np.asarray(a, dtype=mybir.dt.np(BF16))


def _make_common(mask, Wqkv1, bqkv1, Wqkv2, bqkv2, Wnn1, bnn1,
                 zero_bias=False):
    f32 = lambda a: np.ascontiguousarray(np.asarray(a, dtype=np.float32))
    bqkv1, bqkv2, bnn1 = f32(bqkv1), f32(bqkv2), f32(bnn1)
    mT = np.asarray(mask, np.float32)[0, 0].T  # [keys, queries]
    maskTd = np.concatenate(
        [mT[:, :512], mT[:, :512], mT[:, 512:], mT[:, 512:]], axis=1
    )
    common = {
        "maskTd": np.ascontiguousarray(_bf16(maskTd)),
        "Wqkv1": np.ascontiguousarray(_bf16(Wqkv1)),
        "Wqkv2": np.ascontiguousarray(_bf16(Wqkv2)),
        "Wnn1": np.ascontiguousarray(_bf16(Wnn1)),
    }
    if not zero_bias:
        brows = np.stack([bqkv1[2 * DIM :], bqkv2[2 * DIM :], bnn1])
        common.update({
            "bqkv1": bqkv1,
            "bqkv2": bqkv2,
            "brows": np.ascontiguousarray(_bf16(brows)),
        })
    return common


def _bias_free(*biases):
    return all(not np.any(np.asarray(b)) for b in biases)


def kernel(x, mask, Wqkv1, bqkv1, Wqkv2, bqkv2, Wnn1, bnn1):
    from concourse.bass_utils import run_bass_kernel_spmd

    x = np.asarray(x, dtype=np.float32)
    zb = _bias_free(bqkv1, bqkv2, bnn1)
    common = _make_common(mask, Wqkv1, bqkv1, Wqkv2, bqkv2, Wnn1, bnn1,
                          zero_bias=zb)
    in_maps = [
        {"xT": np.ascontiguousarray(_bf16(x[c].T)), **common}
        for c in range(x.shape[0])
    ]
    nc = _get_program(zero_bias=zb)
    res = run_bass_kernel_spmd(nc, in_maps, core_ids=list(range(8)))
    return np.stack([res.results[c]["out"] for c in range(8)]).astype(np.float32)
